# revision 13
# baseline (speedup 1.0000x reference)
"""GCN ConvBNReLU (gnn_message_passing) Trainium2 kernel, 8-core SPMD.

Strategy (graph/data parallel over dst nodes):
  - host: append self-loops, shard edges by dst-owner core, bucket by
    64-node dst windows, sort each bucket's edges by src-half (int16
    gather-index limit), pad groups to 128-edge chunks. Shared static
    layout across cores = max-over-cores chunk counts.
  - device, phase A: every core computes g = dinv_src * (x @ W) for all
    N nodes into two DRAM gather tables (lo/hi 32768 rows each).
  - device, phase B: dma_gather 256B g rows by edge src; per 128-edge
    chunk build a one-hot [128 edges x 64 slots] on DVE (iota==dstloc)
    and matmul-accumulate into the bucket's PSUM tile -> segment sum.
  - device, phase C: scale by dinv_dst, BN stats via ones-matmul column
    sums + AllReduce across 8 cores, normalize + ReLU, write own slice.
"""

import os
import sys

import numpy as np

sys.path.insert(0, "/opt/trn_rl_repo")

import concourse.bacc as bacc  # noqa: E402
import concourse.bass as bass  # noqa: E402
import concourse.mybir as mybir  # noqa: E402
import concourse.tile as tile  # noqa: E402
from concourse.bass_utils import run_bass_kernel_spmd  # noqa: E402

F32 = mybir.dt.float32
I16 = mybir.dt.int16
AF = mybir.ActivationFunctionType
ALU = mybir.AluOpType

CORES = 8
D = 64
W_BUCKET = 64  # dst nodes per aggregation bucket
SEG_CHUNKS = 64  # 8192 gathered rows per dma_gather call
BN_EPS = 1e-5

last_results = None  # BassKernelResults of the most recent run (for test.py)


def _cdiv(a, b):
    return (a + b - 1) // b


def _prep(x, edge_index, n, cores):
    """Host-side sharding/bucketing. Returns plan + per-core arrays."""
    npc = n // cores
    nbuck = npc // W_BUCKET
    n_tables = 2 if n > 32768 else 1
    trows = n // n_tables

    src = np.concatenate(
        [np.asarray(edge_index[0]), np.arange(n, dtype=np.int64)]
    ).astype(np.int64)
    dst = np.concatenate(
        [np.asarray(edge_index[1]), np.arange(n, dtype=np.int64)]
    ).astype(np.int64)
    deg = np.bincount(dst, minlength=n).astype(np.float32)  # incl self-loops

    per_core = []
    counts = np.zeros((cores, n_tables, nbuck), dtype=np.int64)
    for c in range(cores):
        sel = (dst // npc) == c
        s_c = src[sel]
        dl_c = (dst[sel] - c * npc).astype(np.int64)
        half = s_c // trows if n_tables == 2 else np.zeros_like(s_c)
        buck = dl_c // W_BUCKET
        key = half * nbuck + buck
        order = np.argsort(key, kind="stable")
        s_c, dl_c, key = s_c[order], dl_c[order], key[order]
        cnt = np.bincount(key, minlength=n_tables * nbuck)
        counts[c] = cnt.reshape(n_tables, nbuck)
        per_core.append((s_c, dl_c))

    cap = np.maximum.reduce(
        [_cdiv_arr(counts[c]) for c in range(cores)]
    )  # [n_tables, nbuck] chunks per group
    total_chunks = int(cap.sum())
    total_slots = total_chunks * 128

    # chunk stream metadata (shared across cores)
    chunk_bucket = np.zeros(total_chunks, dtype=np.int64)
    chunk_half = np.zeros(total_chunks, dtype=np.int64)
    chunk_first = np.zeros(total_chunks, dtype=bool)
    chunk_last = np.zeros(total_chunks, dtype=bool)
    group_off = {}  # (h, b) -> chunk0
    ct = 0
    for h in range(n_tables):
        for b in range(nbuck):
            k = int(cap[h, b])
            if k == 0:
                continue
            group_off[(h, b)] = ct
            chunk_bucket[ct : ct + k] = b
            chunk_half[ct : ct + k] = h
            chunk_first[ct] = True
            chunk_last[ct + k - 1] = True
            ct += k
    assert ct == total_chunks
    bucket_first_half = {}
    for b in range(nbuck):
        for h in range(n_tables):
            if (h, b) in group_off:
                bucket_first_half[b] = h
                break
        assert b in bucket_first_half, f"bucket {b} has no edges?!"

    # segments: contiguous chunk ranges within one half, <= SEG_CHUNKS
    segments = []  # (half, chunk0, nchunks)
    for h in range(n_tables):
        hsel = np.nonzero(chunk_half == h)[0]
        if len(hsel) == 0:
            continue
        c0 = int(hsel[0])
        cend = int(hsel[-1]) + 1
        cur = c0
        while cur < cend:
            take = min(SEG_CHUNKS, cend - cur)
            # don't split a group across segments unless group > SEG_CHUNKS
            end = cur + take
            if end < cend and not chunk_first[end]:
                while end > cur + 1 and not chunk_first[end]:
                    end -= 1
                if end == cur:  # single giant group; split anyway
                    end = cur + take
            segments.append((h, cur, end - cur))
            cur = end

    # per-core padded slot arrays
    core_inputs = []
    for c in range(cores):
        s_c, dl_c = per_core[c]
        cnt = counts[c]
        src_slots = np.zeros(total_slots, dtype=np.int16)
        dst_slots = np.full(total_slots, float(W_BUCKET), dtype=np.float32)
        pos = 0
        for h in range(n_tables):
            for b in range(nbuck):
                k = int(cap[h, b])
                if k == 0:
                    continue
                m = int(cnt[h, b])
                ct0 = group_off[(h, b)]
                so = ct0 * 128
                src_slots[so : so + m] = (s_c[pos : pos + m] - h * trows).astype(
                    np.int16
                )
                dst_slots[so : so + m] = (dl_c[pos : pos + m] - b * W_BUCKET).astype(
                    np.float32
                )
                pos += m
        assert pos == len(s_c)
        srcidx = np.tile(
            src_slots.reshape(total_slots // 16, 16).T, (8, 1)
        )  # [128, total_slots//16]
        dstloc = dst_slots.reshape(total_chunks, 128).T.copy()  # [128, total_chunks]
        deg_own = (
            deg[c * npc : (c + 1) * npc].reshape(npc // 128, 128).T.copy()
        )  # [128, npairs]
        core_inputs.append(
            {"srcidx": np.ascontiguousarray(srcidx), "dstloc": dstloc, "deg_own": deg_own}
        )

    plan = dict(
        n=n,
        npc=npc,
        nbuck=nbuck,
        npair=npc // 128,
        n_tables=n_tables,
        trows=trows,
        total_chunks=total_chunks,
        total_slots=total_slots,
        chunk_bucket=chunk_bucket,
        chunk_first=chunk_first,
        chunk_last=chunk_last,
        chunk_half=chunk_half,
        bucket_first_half=bucket_first_half,
        segments=segments,
    )
    return plan, core_inputs, deg


def _cdiv_arr(a):
    return -(-a // 128)


def _build(plan, cores):
    """Build the SPMD Tile program (one program, per-core data)."""
    n, npc = plan["n"], plan["npc"]
    npair = plan["npair"]
    n_tables, trows = plan["n_tables"], plan["trows"]
    total_chunks, total_slots = plan["total_chunks"], plan["total_slots"]
    ntile = n // 128  # node tiles (128 nodes each)
    a_chunk = 32 if ntile % 32 == 0 else 1  # node tiles per xT DMA chunk

    nc = bacc.Bacc("TRN2", target_bir_lowering=False, debug=False, num_devices=cores)

    xT = nc.dram_tensor("xT", [D, n], F32, kind="ExternalInput")
    Wt = nc.dram_tensor("W", [D, D], F32, kind="ExternalInput")
    degT = nc.dram_tensor("degT", [128, ntile], F32, kind="ExternalInput")
    deg_own = nc.dram_tensor("deg_own", [128, npair], F32, kind="ExternalInput")
    iota64 = nc.dram_tensor("iota64", [128, W_BUCKET], F32, kind="ExternalInput")
    srcidx_d = nc.dram_tensor(
        "srcidx", [128, total_slots // 16], I16, kind="ExternalInput"
    )
    dstloc_d = nc.dram_tensor("dstloc", [128, total_chunks], F32, kind="ExternalInput")
    gamma_d = nc.dram_tensor("gamma", [1, D], F32, kind="ExternalInput")
    beta_d = nc.dram_tensor("beta", [1, D], F32, kind="ExternalInput")
    y_d = nc.dram_tensor("y", [npc, D], F32, kind="ExternalOutput")

    with tile.TileContext(nc) as tc:
        with (
            tc.tile_pool(name="persist", bufs=1) as pp,
            tc.tile_pool(name="dram", bufs=1, space="DRAM") as dp,
        ):
            # gather source tables must be non-paged DRAM: internal
            # (scratchpad) tiles crash the gather ucode -> ExternalOutput
            gtab = [
                nc.dram_tensor(f"gtab{i}", [trows, D], F32, kind="ExternalOutput")
                for i in range(n_tables)
            ]

            # ---- constants / per-core meta ----
            iota_t = pp.tile([128, W_BUCKET], F32, tag="iota")
            nc.sync.dma_start(iota_t[:], iota64[:])
            deg_t = pp.tile([128, ntile], F32, tag="deg")
            nc.sync.dma_start(deg_t[:], degT[:])
            dinv_t = pp.tile([128, ntile], F32, tag="dinv")
            stmp = pp.tile([128, ntile], F32, tag="stmp")
            nc.scalar.sqrt(stmp[:], deg_t[:])
            nc.vector.reciprocal(dinv_t[:], stmp[:])
            dego_t = pp.tile([128, npair], F32, tag="dego")
            nc.sync.dma_start(dego_t[:], deg_own[:])
            dinvo_t = pp.tile([128, npair], F32, tag="dinvo")
            stmp2 = pp.tile([128, npair], F32, tag="stmp2")
            nc.scalar.sqrt(stmp2[:], dego_t[:])
            nc.vector.reciprocal(dinvo_t[:], stmp2[:])
            srcidx_t = pp.tile([128, total_slots // 16], I16, tag="srcidx")
            nc.sync.dma_start(srcidx_t[:], srcidx_d[:])
            dstloc_t = pp.tile([128, total_chunks], F32, tag="dstloc")
            nc.sync.dma_start(dstloc_t[:], dstloc_d[:])
            w_t = pp.tile([D, D], F32, tag="w")
            nc.sync.dma_start(w_t[:], Wt[:])
            ones_t = pp.tile([128, 1], F32, tag="ones")
            nc.gpsimd.memset(ones_t[:], 1.0)
            gamma_t = pp.tile([1, D], F32, tag="gamma")
            nc.sync.dma_start(gamma_t[:], gamma_d[:])
            beta_t = pp.tile([1, D], F32, tag="beta")
            nc.sync.dma_start(beta_t[:], beta_d[:])

            A_t = pp.tile([128, npair * D], F32, tag="A")

            # ---- phase A: g = dinv * (x @ W) into gather tables ----
            pa = tc.alloc_tile_pool(name="phA", bufs=2)
            pap = tc.alloc_tile_pool(name="phApsum", bufs=4, space="PSUM")
            tiles_per_table = trows // 128
            chunks_per_table = tiles_per_table // a_chunk

            def emit_A(tab):
                if os.environ.get("K_SKIP_A"):
                    return
                for chl in range(chunks_per_table):
                    ch = tab * chunks_per_table + chl
                    xt = pa.tile([D, a_chunk * 128], F32, tag="xt", name="xt")
                    nc.sync.dma_start(
                        xt[:], xT[:, ch * a_chunk * 128 : (ch + 1) * a_chunk * 128]
                    )
                    gst = pa.tile([128, a_chunk * D], F32, tag="gst", name="gst")
                    for t in range(a_chunk):
                        nt = ch * a_chunk + t
                        hp = pap.tile([128, D], F32, tag="hp", name="hp")
                        nc.tensor.matmul(
                            out=hp[:],
                            lhsT=xt[:, t * 128 : (t + 1) * 128],
                            rhs=w_t[:],
                            start=True,
                            stop=True,
                        )
                        if t % 2 == 0:
                            nc.scalar.activation(
                                gst[:, t * D : (t + 1) * D],
                                hp[:],
                                AF.Copy,
                                scale=dinv_t[:, nt : nt + 1],
                            )
                        else:
                            nc.vector.tensor_scalar_mul(
                                gst[:, t * D : (t + 1) * D],
                                hp[:],
                                dinv_t[:, nt : nt + 1],
                            )
                    r0 = chl * a_chunk * 128
                    gview = gtab[tab][r0 : r0 + a_chunk * 128, :].rearrange(
                        "(t p) f -> p t f", p=128
                    )
                    nc.sync.dma_start(
                        gview, gst[:].rearrange("p (t f) -> p t f", f=D)
                    )

            # ---- phase B: gather + one-hot matmul segment-sum ----
            chunk_bucket = plan["chunk_bucket"]
            chunk_first = plan["chunk_first"]
            chunk_last = plan["chunk_last"]
            bucket_first_half = plan["bucket_first_half"]
            pb = tc.alloc_tile_pool(name="phB", bufs=4)
            pboh = tc.alloc_tile_pool(name="phBoh", bufs=6)
            pbp = tc.alloc_tile_pool(name="phBpsum", bufs=4, space="PSUM")
            cur_ps = {}

            def emit_B(half):
                for h, c0, nch in plan["segments"]:
                    if h != half:
                        continue
                    gb = pb.tile([128, SEG_CHUNKS * D], F32, tag="gb")
                    nidx = nch * 128
                    out_ap = gb[:].rearrange("p (c f) -> p c f", f=D)[:, :nch, :]
                    if os.environ.get("K_NO_GATHER"):
                        nc.gpsimd.memset(gb[:], 0.0)
                    else:
                        nc.gpsimd.dma_gather(
                            out_ap,
                            gtab[h][:],
                            srcidx_t[:, c0 * 8 : (c0 + nch) * 8],
                            nidx,
                            nidx,
                            D,
                            single_packet=False,
                        )
                    for j in range(nch):
                        ct = c0 + j
                        b = int(chunk_bucket[ct])
                        o = (b % 2) * 64
                        if chunk_first[ct]:
                            cur_ps[b] = pbp.tile([128, D], F32, tag="agg", name="agg")
                        oh = pboh.tile([128, W_BUCKET], F32, tag="oh")
                        nc.vector.tensor_tensor(
                            out=oh[:],
                            in0=dstloc_t[:, ct : ct + 1].to_broadcast(
                                [128, W_BUCKET]
                            ),
                            in1=iota_t[:],
                            op=ALU.is_equal,
                        )
                        nc.tensor.matmul(
                            out=cur_ps[b][o : o + 64, :],
                            lhsT=oh[:],
                            rhs=gb[:, j * D : (j + 1) * D],
                            start=bool(chunk_first[ct]),
                            stop=bool(chunk_last[ct]),
                        )
                        if chunk_last[ct]:
                            av = A_t[o : o + 64, (b // 2) * D : (b // 2 + 1) * D]
                            psv = cur_ps[b][o : o + 64, :]
                            if bucket_first_half[b] == h:
                                nc.scalar.activation(av, psv, AF.Copy)
                            else:
                                nc.vector.tensor_add(av, av, psv)
                            del cur_ps[b]

            emit_A(0)
            emit_B(0)
            if n_tables == 2:
                emit_A(1)
                emit_B(1)
            for _pool in (pbp, pboh, pb, pap, pa):
                _pool.release()

            # ---- phase C: dinv_dst scale, BN stats, allreduce, norm+relu ----
            with (
                tc.tile_pool(name="phC", bufs=4) as pc,
                tc.tile_pool(name="phCpsum", bufs=2, space="PSUM") as pcp,
            ):
                ssum_ps = pcp.tile([1, D], F32, tag="ssum")
                ssq_ps = pcp.tile([1, D], F32, tag="ssq")
                dinvsq_t = pp.tile([128, npair], F32, tag="dinvsq")
                nc.vector.tensor_mul(dinvsq_t[:], dinvo_t[:], dinvo_t[:])
                for p in range(npair):
                    av = A_t[:, p * D : (p + 1) * D]
                    sq = pc.tile([128, D], F32, tag="sq")
                    nc.scalar.square(sq[:], av)
                    nc.tensor.matmul(
                        out=ssum_ps[:],
                        lhsT=dinvo_t[:, p : p + 1],
                        rhs=av,
                        start=(p == 0),
                        stop=(p == npair - 1),
                    )
                    nc.tensor.matmul(
                        out=ssq_ps[:],
                        lhsT=dinvsq_t[:, p : p + 1],
                        rhs=sq[:],
                        start=(p == 0),
                        stop=(p == npair - 1),
                    )
                stats = pc.tile([1, 2 * D], F32, tag="stats")
                nc.vector.tensor_copy(stats[0:1, 0:D], ssum_ps[:])
                nc.vector.tensor_copy(stats[0:1, D : 2 * D], ssq_ps[:])
                cc_in = dp.tile([1, 2 * D], F32, tag="ccin", name="ccin")
                cc_out = dp.tile([1, 2 * D], F32, tag="ccout", name="ccout")
                nc.sync.dma_start(cc_in[:], stats[:])
                if os.environ.get("K_NO_CC"):
                    nc.sync.dma_start(cc_out[:], cc_in[:])
                else:
                    nc.gpsimd.collective_compute(
                        "AllReduce",
                        ALU.add,
                        replica_groups=[list(range(cores))],
                        ins=[cc_in.opt()],
                        outs=[cc_out.opt()],
                    )
                gstats = pc.tile([1, 2 * D], F32, tag="gstats")
                nc.sync.dma_start(gstats[:], cc_out[:])

                mean_t = pc.tile([1, D], F32, tag="mean")
                nc.vector.tensor_scalar_mul(mean_t[:], gstats[0:1, 0:D], 1.0 / n)
                ex2_t = pc.tile([1, D], F32, tag="ex2")
                nc.vector.tensor_scalar_mul(ex2_t[:], gstats[0:1, D : 2 * D], 1.0 / n)
                msq_t = pc.tile([1, D], F32, tag="msq")
                nc.vector.tensor_mul(msq_t[:], mean_t[:], mean_t[:])
                var_t = pc.tile([1, D], F32, tag="var")
                nc.vector.tensor_sub(var_t[:], ex2_t[:], msq_t[:])
                vare_t = pc.tile([1, D], F32, tag="vare")
                nc.vector.tensor_scalar_add(vare_t[:], var_t[:], BN_EPS)
                std_t = pc.tile([1, D], F32, tag="std")
                nc.scalar.sqrt(std_t[:], vare_t[:])
                istd_t = pc.tile([1, D], F32, tag="istd")
                nc.vector.reciprocal(istd_t[:], std_t[:])
                scf_t = pc.tile([1, D], F32, tag="scf")
                nc.vector.tensor_mul(scf_t[:], gamma_t[:], istd_t[:])
                mtmp_t = pc.tile([1, D], F32, tag="mtmp")
                nc.vector.tensor_mul(mtmp_t[:], mean_t[:], scf_t[:])
                shf_t = pc.tile([1, D], F32, tag="shf")
                nc.vector.tensor_sub(shf_t[:], beta_t[:], mtmp_t[:])
                scb_t = pp.tile([128, D], F32, tag="scb")
                nc.gpsimd.partition_broadcast(scb_t[:], scf_t[:])
                shb_t = pp.tile([128, D], F32, tag="shb")
                nc.gpsimd.partition_broadcast(shb_t[:], shf_t[:])

                for p in range(npair):
                    av = A_t[:, p * D : (p + 1) * D]
                    t1 = pc.tile([128, D], F32, tag="t1")
                    nc.vector.scalar_tensor_tensor(
                        t1[:], av, dinvo_t[:, p : p + 1], scb_t[:],
                        op0=ALU.mult, op1=ALU.mult,
                    )
                    nc.vector.tensor_add(t1[:], t1[:], shb_t[:])
                    nc.scalar.activation(av, t1[:], AF.Relu)
                nc.sync.dma_start(
                    y_d[:].rearrange("(p q) f -> q p f", q=128),
                    A_t[:].rearrange("q (p f) -> q p f", f=D),
                )

    nc.compile()
    return nc


def _run(x, edge_index, W, b, gamma, beta, cores=CORES, trace=False):
    global last_results
    n, d = x.shape
    assert d == D and n % (cores * 128) == 0
    plan, core_inputs, _deg = _prep(x, edge_index, n, cores)
    nc = _build(plan, cores)

    npc = plan["npc"]
    ntile = n // 128
    deg = _deg
    shared = {
        "xT": np.ascontiguousarray(np.asarray(x, dtype=np.float32).T),
        "W": np.asarray(W, dtype=np.float32),
        "degT": np.ascontiguousarray(deg.reshape(ntile, 128).T),
        "iota64": np.tile(np.arange(W_BUCKET, dtype=np.float32), (128, 1)),
        "gamma": np.asarray(gamma, dtype=np.float32).reshape(1, D),
        "beta": np.asarray(beta, dtype=np.float32).reshape(1, D),
    }
    in_maps = []
    for c in range(cores):
        m = dict(shared)
        m.update(core_inputs[c])
        in_maps.append(m)

    import time as _time

    t0 = _time.time()
    try:
        res = run_bass_kernel_spmd(
            nc, in_maps, core_ids=list(range(cores)), trace=trace
        )
    except ModuleNotFoundError:
        res = run_bass_kernel_spmd(
            nc, in_maps, core_ids=list(range(cores)), trace=False
        )
    res.wallclock_exec_s = _time.time() - t0
    last_results = res
    y = np.concatenate([res.results[c]["y"] for c in range(cores)], axis=0)
    return y


def kernel(**inputs):
    return _run(
        np.asarray(inputs["x"], dtype=np.float32),
        np.asarray(inputs["edge_index"]),
        inputs["W"],
        inputs["b"],
        inputs["gamma"],
        inputs["beta"],
        trace=bool(int(os.environ.get("KERNEL_TRACE", "0"))),
    )
